# revision 26
# baseline (speedup 1.0000x reference)
"""Causal self-attention (B=2, S=2048, D=1024, H=16, Dh=64) on 8 trn2 cores.

Sharding: data-parallel over batch (2 groups of 4 cores) x tensor-parallel over
heads (4 heads/core). Each core computes its heads' attention and a partial
c_proj product; the host sums the 4 bf16 partials per batch (in f32) and adds
b_proj.

All matmuls are bf16 (PSUM accumulates f32; fp8 was tested on HW and is
numerically dead here: e4m3 adds ~2.7% Frobenius noise per quantized tensor,
which does not average down with contraction size). Design notes:
  - Schedule: all QKV upfront (PE-dense stream), then attention q-blocks in
    DESCENDING order with the TWO HEAD-PAIRS' pipelines interleaved tick by
    tick (independent work hides exp latency and halves pipeline
    drain/fill boundaries; phat is per head-pair for this). The projection
    of each q-block is drained one tick at a time as filler into the
    following (shorter, exp-latency-bound) q-blocks.
  - The attention-output normalization has no transposes: PV output
    po = o^T [128, 512] per head (row 0 = softmax denominator via a ones
    column in V-hat -- it must sit at psum partition 0 because
    reciprocal_approx_fast misreads psum inputs at non-zero partition
    offsets; V sits at rows 64:127 since psum reads need 32-aligned
    starts). 1/den via reciprocal_approx_fast (DVE), partition_broadcast
    across the 64 feature rows (GpSimd), one scale+cast mul per head (DVE)
    into the proj lhsT layout.
  - Input DMAs are issued in dt-pair batches from both the sync and gpsimd
    queues (descriptor gen is ~0.65us per dma_start and was gating startup).
  - PSUM is exactly 8 banks: scores 2x[128,1024] (4) + po-ring 2x (4).
    Matmul outputs never cross a 2KB bank; stop=True only on the last
    write of each bank (it closes the whole bank's accumulation group).
  - HW timing is noisy (~+-10%): the chip sits in a per-run DVFS state
    (216ns vs 427ns per 512-row matmul); a PE "warmup" cannot lift it.
"""

import os
import sys

for _p in ("/opt/trn_rl_repo", "/root/.axon_site/_ro/trn_rl_repo"):
    if os.path.isdir(_p) and _p not in sys.path:
        sys.path.insert(0, _p)

import numpy as np
import ml_dtypes

import concourse.bacc as bacc
import concourse.tile as tile
from concourse import mybir
from concourse.bass_utils import run_bass_kernel_spmd

F32 = mybir.dt.float32
BF16 = mybir.dt.bfloat16
EXP = mybir.ActivationFunctionType.Exp

B, S, D, H, DH = 2, 2048, 1024, 16, 64
HC = 4          # heads per core
EQK = 512       # q+k weight cols per core
EV = 256        # v weight cols per core
ND = D // 128   # 8 d-tiles
NS = S // 128   # 16 k-tiles of 128
NQ = S // 512   # 4 s/q-blocks of 512


def build_nc():
    nc = bacc.Bacc("TRN2", target_bir_lowering=False, debug=False)

    xT = nc.dram_tensor("xT", [D, S], BF16, kind="ExternalInput").ap()
    wqk = nc.dram_tensor("wqk", [D, EQK], BF16, kind="ExternalInput").ap()
    bqk = nc.dram_tensor("bqk", [128, 4], F32, kind="ExternalInput").ap()
    wv = nc.dram_tensor("wv", [D, EV], BF16, kind="ExternalInput").ap()
    bv = nc.dram_tensor("bv", [1, EV], BF16, kind="ExternalInput").ap()
    wp = nc.dram_tensor("wp", [EV, D], BF16, kind="ExternalInput").ap()
    masks = nc.dram_tensor("masks", [128, 4, 512], BF16, kind="ExternalInput").ap()
    y = nc.dram_tensor("y", [S, D], BF16, kind="ExternalOutput").ap()

    with tile.TileContext(nc) as tc:
        _emit(nc, tc, xT, wqk, bqk, wv, bv, wp, masks, y)
    nc.compile()
    return nc


def _emit(nc, tc, xT, wqk, bqk, wv, bv, wp, masks, y, dbg=None):
    from contextlib import ExitStack

    with ExitStack() as top:
        consts = top.enter_context(tc.tile_pool(name="consts", bufs=1))
        acts = top.enter_context(tc.tile_pool(name="acts", bufs=1))
        xt_pool = top.enter_context(tc.tile_pool(name="xt", bufs=2))
        ps_sc = top.enter_context(tc.tile_pool(name="ps_sc", bufs=4, space="PSUM"))
        ps_po = top.enter_context(tc.tile_pool(name="ps_po", bufs=2, space="PSUM"))
        norm_pool = top.enter_context(tc.tile_pool(name="norm", bufs=2))
        ysb_pool = top.enter_context(tc.tile_pool(name="ysb", bufs=3))

        ones_col = consts.tile([1, 128], BF16)
        nc.vector.memset(ones_col, 1.0)
        masks_sb = consts.tile([128, 4, 512], BF16)
        bqk_sb = consts.tile([128, 4], F32)
        bv_sb = consts.tile([1, EV], BF16)
        wqk_sb = consts.tile([128, ND, EQK], BF16)
        wv_sb = consts.tile([128, ND, EV], BF16)
        wp_sb = consts.tile([128, 2, D], BF16)

        # persistent activations
        qt_sb = [acts.tile([128, S], BF16, tag=f"qt{hp}", name=f"qt{hp}") for hp in range(2)]
        kt_sb = [acts.tile([128, S], BF16, tag=f"kt{hp}", name=f"kt{hp}") for hp in range(2)]
        # [1 | 0-pad | V]: col 0 = ones (softmax denominator via PV matmul;
        # it must land on psum partition 0 -- reciprocal_approx_fast misreads
        # psum inputs at non-zero partition offsets), V at cols 64:128 so the
        # o rows sit at psum partition 64 (psum reads need 32-aligned starts)
        vhat = acts.tile([128, HC, NS, 128], BF16, tag="vhat")
        phat = [[acts.tile([128, NS, 512], BF16, tag=f"phat{hp}_{h}", name=f"phat{hp}_{h}")
                 for h in range(2)] for hp in range(2)]
        ot_sb = acts.tile([128, 2, S], BF16, tag="ot_sb")  # proj lhsT [feat, s]

        xt_tiles = [xt_pool.tile([128, ND, 512], BF16, tag="xt", name=f"xt{sb}")
                    for sb in range(NQ)]

        # Startup is DMA-descriptor-issue-bound (~0.65us per dma_start on a
        # sequencer). Issue the first QKV block's inputs first, batched in
        # dt-pairs, split across the sync AND gpsimd queues so descriptor
        # generation runs in parallel; everything else in first-use order.
        def pair_dma(eng, dst, src, t2, c0=None, c1=None):
            s = src[256 * t2:256 * (t2 + 1), :] if c0 is None else \
                src[256 * t2:256 * (t2 + 1), c0:c1]
            eng.dma_start(dst[:, 2 * t2:2 * (t2 + 1), :],
                          s.rearrange("(i p) e -> p i e", p=128))

        for t2 in range(4):
            pair_dma(nc.sync, wqk_sb, wqk, t2)
            pair_dma(nc.gpsimd, xt_tiles[0], xT, t2, 0, 512)
        for t2 in range(2):
            nc.sync.dma_start(
                wv_sb[:, 4 * t2:4 * (t2 + 1), :],
                wv[512 * t2:512 * (t2 + 1), :].rearrange("(i p) e -> p i e", p=128))
        nc.gpsimd.dma_start(bqk_sb[:], bqk[:])
        nc.gpsimd.dma_start(bv_sb[:], bv[:])
        for t2 in range(4):
            pair_dma(nc.sync, xt_tiles[1], xT, t2, 512, 1024)
        nc.gpsimd.dma_start(
            wp_sb[:], wp[:].rearrange("(i p) e -> p i e", p=128))
        nc.sync.dma_start(masks_sb[:], masks[:])
        for sb in (2, 3):
            for t2 in range(2):
                nc.sync.dma_start(
                    xt_tiles[sb][:, 4 * t2:4 * (t2 + 1), :],
                    xT[512 * t2:512 * (t2 + 1), 512 * sb:512 * (sb + 1)]
                    .rearrange("(i p) e -> p i e", p=128))

        nc.gpsimd.memset(vhat[:, :, :, 0:1], 1.0)
        nc.gpsimd.memset(vhat[:, :, :, 1:DH], 0.0)
        # exp skips the fully-masked prefix of diagonal tiles; the mask
        # multiply zeroes those bytes, which needs them finite (0*NaN=NaN)
        for hp in range(2):
            for h in range(2):
                nc.gpsimd.memset(phat[hp][h][:], 0.0)

        def emit_qk(sb):
            """QK projections for s-block sb (po psum ring)."""
            xt = xt_tiles[sb]
            sl = slice(512 * sb, 512 * (sb + 1))
            for hp in range(2):
                pqk = ps_po.tile([128, 1024], F32, tag="po", name=f"pqk{sb}_{hp}")
                for dt in range(ND):
                    for e in range(2):
                        et = 2 * hp + e
                        nc.tensor.matmul(
                            pqk[:, 512 * e:512 * (e + 1)],
                            wqk_sb[:, dt, 128 * et:128 * (et + 1)],
                            xt[:, dt, :],
                            start=(dt == 0), stop=(dt == ND - 1),
                        )
                    if dt == 3:
                        yield
                nc.vector.tensor_scalar_add(
                    qt_sb[hp][:, sl], pqk[:, 0:512], bqk_sb[:, 2 * hp:2 * hp + 1])
                nc.vector.tensor_scalar_add(
                    kt_sb[hp][:, sl], pqk[:, 512:1024], bqk_sb[:, 2 * hp + 1:2 * hp + 2])
                yield

        def emit_v(sb):
            """V projection for s-block sb (po-ring psum slot, 2 banks)."""
            xt = xt_tiles[sb]
            pv = ps_po.tile([128, 1024], F32, tag="po", name=f"pv{sb}")
            for dt in range(ND):
                for st in range(4):
                    # start only on the first region of each psum bank: start
                    # zeroes the whole 2KB-aligned region, so odd-st halves
                    # accumulate onto the pending-zero left by even st
                    nc.tensor.matmul(
                        pv[:, 256 * st:256 * (st + 1)],
                        xt[:, dt, 128 * st:128 * (st + 1)],
                        wv_sb[:, dt, :],
                        start=(dt == 0 and st % 2 == 0), stop=False,
                    )
                if dt == 3:
                    yield
            for st in range(4):
                # stop only on the second half of each 2KB psum bank: stop
                # closes the whole bank's accumulation group
                nc.tensor.matmul(
                    pv[:, 256 * st:256 * (st + 1)],
                    ones_col[:, 0:128],
                    bv_sb[:],
                    start=False, stop=(st % 2 == 1),
                )
            for st in range(4):
                nc.scalar.copy(
                    vhat[:, :, 4 * sb + st, DH:2 * DH],
                    pv[:, 256 * st:256 * (st + 1)].rearrange("p (h e) -> p h e", h=HC),
                )
            yield

        def emit_attn(hp, qt):
            ph = phat[hp]
            """Scores+exp+PV for head-pair hp, q-block qt; yields after each
            pipeline step so the caller can weave in filler PE work."""
            nkt = 4 * (qt + 1)
            ngrp = nkt // 2
            po = ps_po.tile([128, 2, 512], F32, tag="po", name=f"po{hp}_{qt}")

            def emit_scores(g):
                # 1-bank psc tiles per (h, kt): exp consumes and frees each
                # half as soon as its matmul stops, so the other head-pair's
                # scores never wait on a whole-group drain
                o0 = 2 * g - 4 * qt  # diag-band offset of tile 2g (>=0 in band)
                for h in range(2):
                    sl = slice(64 * h, 64 * (h + 1))
                    psc = [ps_sc.tile([128, 512], F32, tag="sc",
                                      name=f"sc{hp}_{qt}_{g}_{h}_{j}")
                           for j in range(2)]
                    for j in range(2):
                        kt = 2 * g + j
                        nc.tensor.matmul(
                            psc[j][:],
                            kt_sb[hp][sl, 128 * kt:128 * (kt + 1)],
                            qt_sb[hp][sl, 512 * qt:512 * (qt + 1)],
                            start=True, stop=True,
                        )
                    for j in range(2):
                        if o0 < 0:
                            nc.scalar.activation(
                                ph[h][:, 2 * g + j, :], psc[j][:],
                                EXP, scale=0.125,
                            )
                        else:
                            nc.scalar.activation(
                                ph[h][:, 2 * g + j, 128 * (o0 + j):512],
                                psc[j][:, 128 * (o0 + j):512],
                                EXP, scale=0.125,
                            )
                            # per-tile mask so PV of tile 2g needn't wait
                            # for tile 2g+1's exp
                            nc.vector.tensor_mul(
                                ph[h][:, 2 * g + j, :],
                                ph[h][:, 2 * g + j, :],
                                masks_sb[:, o0 + j, :])

            def emit_pv(g, h):
                for j in range(2):
                    kt = 2 * g + j
                    nc.tensor.matmul(
                        po[:, h, :],
                        vhat[:, 2 * hp + h, kt, :],
                        ph[h][:, kt, :],
                        start=(kt == 0), stop=(kt == nkt - 1),
                    )

            # software pipeline: pv lags scores by 2 groups
            for g in range(ngrp):
                emit_scores(g)
                yield "grp"
                if g >= 2:
                    emit_pv(g - 2, 0)
                    emit_pv(g - 2, 1)
                    yield "grp"
            for g in range(max(0, ngrp - 2), ngrp):
                emit_pv(g, 0)
                emit_pv(g, 1)
                yield "tail"

            # normalization, no transposes: 1/den via fast-approx reciprocal
            # (DVE, free-size-bound), po copied to bf16 (ACT) freeing the psum
            # slot early, 1/den broadcast across the 64 feature partitions on
            # the idle GpSimd engine, then one scale+cast mul per head into
            # the proj lhsT layout.
            rec = norm_pool.tile([1, 2, 512], F32, tag="rec")
            pob = norm_pool.tile([DH, 2, 512], BF16, tag="pob")
            recb = norm_pool.tile([64, 2, 512], F32, tag="recb")
            nc.vector.reciprocal_approx_fast(rec[:], po[0:1, :, :])
            nc.vector.tensor_copy(pob[:], po[DH:2 * DH, :, :])
            nc.gpsimd.partition_broadcast(recb[:], rec[:])
            yield "tail"
            sl = slice(512 * qt, 512 * (qt + 1))
            for h in range(2):
                nc.vector.tensor_mul(
                    ot_sb[64 * h:64 * (h + 1), hp, sl], pob[:, h, :], recb[:, h, :])
                yield "norm"

        def emit_proj(qt, tail=False):
            """Projection for q-block qt (after both head-pairs normalized)."""
            for st in range(4):
                s0 = 512 * qt + 128 * st
                py = [ps_sc.tile([128, 512], F32, tag="sc", name=f"py{qt}_{st}_{nt}")
                      for nt in range(2)]
                # ft-outer: consecutive matmuls share the stationary ot tile,
                # skipping a ~100ns weight reload per matmul
                for ft in range(2):
                    for nt in range(2):
                        nc.tensor.matmul(
                            py[nt][:],
                            ot_sb[:, ft, s0:s0 + 128],
                            wp_sb[:, ft, 512 * nt:512 * (nt + 1)],
                            start=(ft == 0), stop=(ft == 1),
                        )
                ysb = ysb_pool.tile([128, 1024], BF16, tag="ysb", name=f"ysb{qt}_{st}")
                eng = nc.scalar.copy if (tail and st % 2 == 1) else nc.vector.tensor_copy
                eng(ysb[:, 0:512], py[0][:])
                yield
                eng(ysb[:, 512:1024], py[1][:])
                nc.sync.dma_start(y[s0:s0 + 128, :], ysb[:])
                yield

        # ---------------- schedule ----------------
        # All QKV upfront (PE-dense stream), then attention q-blocks in
        # DESCENDING order: the long streams (qt=3,2) run first with no
        # filler needed, and the short exp-latency-bound blocks (qt=1,0)
        # at the end are filled by the accumulated projection work.
        for sb in range(NQ):
            for _ in emit_qk(sb):
                pass
            for _ in emit_v(sb):
                pass

        fillerP = []

        def drain(queue, n=-1):
            while queue and n != 0:
                try:
                    next(queue[0])
                    n -= 1
                except StopIteration:
                    queue.pop(0)

        for qt in (3, 2, 1, 0):
            gens = [emit_attn(0, qt), emit_attn(1, qt)]
            while gens:
                for g in list(gens):
                    try:
                        next(g)
                    except StopIteration:
                        gens.remove(g)
                        continue
                    drain(fillerP, 1)
            fillerP.append(emit_proj(qt, tail=(qt == 0)))
        drain(fillerP)

        if dbg is not None:
            dqt, dkt, dvh, dph, dot = dbg
            for hp in range(2):
                nc.sync.dma_start(dqt[hp], qt_sb[hp][:])
                nc.sync.dma_start(dkt[hp], kt_sb[hp][:])
                nc.sync.dma_start(dph[hp], phat[hp][0][:])
            nc.sync.dma_start(dvh[:], vhat[:])
            nc.sync.dma_start(dot[:], ot_sb[:])


_NC = None


def _get_nc():
    global _NC
    if _NC is None:
        _NC = build_nc()
    return _NC


def _make_masks():
    i = np.arange(128)[:, None]
    j = np.arange(512)[None, :]
    m = np.stack([(i + 128 * o <= j) for o in range(4)], axis=1)  # [128, 4, 512]
    return m.astype(ml_dtypes.bfloat16)


def _in_maps(x, W_attn, b_attn, W_proj):
    masks = _make_masks()
    bf = ml_dtypes.bfloat16
    maps = []
    for c in range(8):
        b, g = c // 4, c % 4
        heads = [4 * g + i for i in range(HC)]
        qc = [W_attn[:, 64 * h:64 * (h + 1)] for h in heads]
        kc = [W_attn[:, D + 64 * h:D + 64 * (h + 1)] for h in heads]
        vc = [W_attn[:, 2 * D + 64 * h:2 * D + 64 * (h + 1)] for h in heads]
        bq = [b_attn[64 * h:64 * (h + 1)] for h in heads]
        bk = [b_attn[D + 64 * h:D + 64 * (h + 1)] for h in heads]
        bvs = [b_attn[2 * D + 64 * h:2 * D + 64 * (h + 1)] for h in heads]
        # et order: 0 -> Q hp0, 1 -> K hp0, 2 -> Q hp1, 3 -> K hp1
        wqk_c = np.ascontiguousarray(np.concatenate(
            [qc[0], qc[1], kc[0], kc[1], qc[2], qc[3], kc[2], kc[3]], axis=1)).astype(bf)
        bqk_c = np.concatenate(
            [bq[0], bq[1], bk[0], bk[1], bq[2], bq[3], bk[2], bk[3]])
        bqk_c = np.ascontiguousarray(bqk_c.reshape(4, 128).T).astype(np.float32)
        wv_c = np.ascontiguousarray(np.concatenate(vc, axis=1)).astype(bf)
        bv_c = np.ascontiguousarray(np.concatenate(bvs).reshape(1, EV)).astype(bf)
        wp_c = np.ascontiguousarray(W_proj[EV * g:EV * (g + 1), :]).astype(bf)
        xT_c = np.ascontiguousarray(x[b].T).astype(bf)
        maps.append({
            "xT": xT_c, "wqk": wqk_c, "bqk": bqk_c, "wv": wv_c,
            "bv": bv_c, "wp": wp_c, "masks": masks,
        })
    return maps


def _gather(results, b_proj):
    y = np.empty((B, S, D), np.float32)
    for b in range(B):
        acc = results[4 * b]["y"].astype(np.float32)
        for g in range(1, 4):
            acc = acc + results[4 * b + g]["y"].astype(np.float32)
        y[b] = acc + b_proj[None, :]
    return y


def run(x, W_attn, b_attn, W_proj, b_proj, trace=False):
    x = np.asarray(x, np.float32)
    W_attn = np.asarray(W_attn, np.float32)
    b_attn = np.asarray(b_attn, np.float32)
    W_proj = np.asarray(W_proj, np.float32)
    b_proj = np.asarray(b_proj, np.float32)
    nc = _get_nc()
    res = run_bass_kernel_spmd(nc, _in_maps(x, W_attn, b_attn, W_proj),
                               core_ids=list(range(8)), trace=trace)
    return _gather(res.results, b_proj), res


def kernel(x, W_attn, b_attn, W_proj, b_proj):
    out, _ = run(x, W_attn, b_attn, W_proj, b_proj)
    return out



# revision 27
# speedup vs baseline: 1.0188x; 1.0188x over previous
"""Causal self-attention (B=2, S=2048, D=1024, H=16, Dh=64) on 8 trn2 cores.

Sharding: data-parallel over batch (2 groups of 4 cores) x tensor-parallel over
heads (4 heads/core). Each core computes its heads' attention and a partial
c_proj product; the host sums the 4 bf16 partials per batch (in f32) and adds
b_proj.

All matmuls are bf16 (PSUM accumulates f32; fp8 was tested on HW and is
numerically dead here: e4m3 adds ~2.7% Frobenius noise per quantized tensor,
which does not average down with contraction size). Design notes:
  - Schedule: all QKV upfront (PE-dense stream), then attention q-blocks in
    DESCENDING order with the TWO HEAD-PAIRS' pipelines interleaved tick by
    tick (independent work hides exp latency and halves pipeline
    drain/fill boundaries; phat is per head-pair for this). The projection
    of each q-block is drained one tick at a time as filler into the
    following (shorter, exp-latency-bound) q-blocks.
  - The attention-output normalization has no transposes: PV output
    po = o^T [128, 512] per head (row 0 = softmax denominator via a ones
    column in V-hat -- it must sit at psum partition 0 because
    reciprocal_approx_fast misreads psum inputs at non-zero partition
    offsets; V sits at rows 64:127 since psum reads need 32-aligned
    starts). 1/den via reciprocal_approx_fast (DVE), partition_broadcast
    across the 64 feature rows (GpSimd), one scale+cast mul per head (DVE)
    into the proj lhsT layout.
  - Input DMAs are issued in dt-pair batches from both the sync and gpsimd
    queues (descriptor gen is ~0.65us per dma_start and was gating startup).
  - PSUM is exactly 8 banks: scores 2x[128,1024] (4) + po-ring 2x (4).
    Matmul outputs never cross a 2KB bank; stop=True only on the last
    write of each bank (it closes the whole bank's accumulation group).
  - HW timing is noisy (~+-10%): the chip sits in a per-run DVFS state
    (216ns vs 427ns per 512-row matmul); a PE "warmup" cannot lift it.
"""

import os
import sys

for _p in ("/opt/trn_rl_repo", "/root/.axon_site/_ro/trn_rl_repo"):
    if os.path.isdir(_p) and _p not in sys.path:
        sys.path.insert(0, _p)

import numpy as np
import ml_dtypes

import concourse.bacc as bacc
import concourse.tile as tile
from concourse import mybir
from concourse.bass_utils import run_bass_kernel_spmd

F32 = mybir.dt.float32
BF16 = mybir.dt.bfloat16
EXP = mybir.ActivationFunctionType.Exp

B, S, D, H, DH = 2, 2048, 1024, 16, 64
HC = 4          # heads per core
EQK = 512       # q+k weight cols per core
EV = 256        # v weight cols per core
ND = D // 128   # 8 d-tiles
NS = S // 128   # 16 k-tiles of 128
NQ = S // 512   # 4 s/q-blocks of 512


def build_nc():
    nc = bacc.Bacc("TRN2", target_bir_lowering=False, debug=False)

    xT = nc.dram_tensor("xT", [D, S], BF16, kind="ExternalInput").ap()
    wqk = nc.dram_tensor("wqk", [D, EQK], BF16, kind="ExternalInput").ap()
    bqk = nc.dram_tensor("bqk", [128, 4], F32, kind="ExternalInput").ap()
    wv = nc.dram_tensor("wv", [D, EV], BF16, kind="ExternalInput").ap()
    bv = nc.dram_tensor("bv", [1, EV], BF16, kind="ExternalInput").ap()
    wp = nc.dram_tensor("wp", [EV, D], BF16, kind="ExternalInput").ap()
    masks = nc.dram_tensor("masks", [128, 4, 512], BF16, kind="ExternalInput").ap()
    y = nc.dram_tensor("y", [S, D], BF16, kind="ExternalOutput").ap()

    with tile.TileContext(nc) as tc:
        _emit(nc, tc, xT, wqk, bqk, wv, bv, wp, masks, y)
    nc.compile()
    return nc


def _emit(nc, tc, xT, wqk, bqk, wv, bv, wp, masks, y, dbg=None):
    from contextlib import ExitStack

    with ExitStack() as top:
        consts = top.enter_context(tc.tile_pool(name="consts", bufs=1))
        acts = top.enter_context(tc.tile_pool(name="acts", bufs=1))
        xt_pool = top.enter_context(tc.tile_pool(name="xt", bufs=2))
        ps_sc = top.enter_context(tc.tile_pool(name="ps_sc", bufs=4, space="PSUM"))
        ps_po = top.enter_context(tc.tile_pool(name="ps_po", bufs=2, space="PSUM"))
        norm_pool = top.enter_context(tc.tile_pool(name="norm", bufs=2))
        ysb_pool = top.enter_context(tc.tile_pool(name="ysb", bufs=3))

        ones_col = consts.tile([1, 128], BF16)
        nc.vector.memset(ones_col, 1.0)
        masks_sb = consts.tile([128, 4, 512], BF16)
        bqk_sb = consts.tile([128, 4], F32)
        bv_sb = consts.tile([1, EV], BF16)
        wqk_sb = consts.tile([128, ND, EQK], BF16)
        wv_sb = consts.tile([128, ND, EV], BF16)
        wp_sb = consts.tile([128, 2, D], BF16)

        # persistent activations
        qt_sb = [acts.tile([128, S], BF16, tag=f"qt{hp}", name=f"qt{hp}") for hp in range(2)]
        kt_sb = [acts.tile([128, S], BF16, tag=f"kt{hp}", name=f"kt{hp}") for hp in range(2)]
        # [1 | 0-pad | V]: col 0 = ones (softmax denominator via PV matmul;
        # it must land on psum partition 0 -- reciprocal_approx_fast misreads
        # psum inputs at non-zero partition offsets), V at cols 64:128 so the
        # o rows sit at psum partition 64 (psum reads need 32-aligned starts)
        vhat = acts.tile([128, HC, NS, 128], BF16, tag="vhat")
        phat = [[acts.tile([128, NS, 512], BF16, tag=f"phat{hp}_{h}", name=f"phat{hp}_{h}")
                 for h in range(2)] for hp in range(2)]
        ot_sb = acts.tile([128, 2, S], BF16, tag="ot_sb")  # proj lhsT [feat, s]

        xt_tiles = [xt_pool.tile([128, ND, 512], BF16, tag="xt", name=f"xt{sb}")
                    for sb in range(NQ)]

        # Startup is DMA-descriptor-issue-bound (~0.65us per dma_start on a
        # sequencer). Issue the first QKV block's inputs first, batched in
        # dt-pairs, split across the sync AND gpsimd queues so descriptor
        # generation runs in parallel; everything else in first-use order.
        def pair_dma(eng, dst, src, t2, c0=None, c1=None):
            s = src[256 * t2:256 * (t2 + 1), :] if c0 is None else \
                src[256 * t2:256 * (t2 + 1), c0:c1]
            eng.dma_start(dst[:, 2 * t2:2 * (t2 + 1), :],
                          s.rearrange("(i p) e -> p i e", p=128))

        for t2 in range(4):
            pair_dma(nc.sync, wqk_sb, wqk, t2)
            pair_dma(nc.gpsimd, xt_tiles[0], xT, t2, 0, 512)
        for t2 in range(2):
            nc.sync.dma_start(
                wv_sb[:, 4 * t2:4 * (t2 + 1), :],
                wv[512 * t2:512 * (t2 + 1), :].rearrange("(i p) e -> p i e", p=128))
        nc.gpsimd.dma_start(bqk_sb[:], bqk[:])
        nc.gpsimd.dma_start(bv_sb[:], bv[:])
        for t2 in range(4):
            pair_dma(nc.sync, xt_tiles[1], xT, t2, 512, 1024)
        nc.gpsimd.dma_start(
            wp_sb[:], wp[:].rearrange("(i p) e -> p i e", p=128))
        nc.sync.dma_start(masks_sb[:], masks[:])
        for sb in (2, 3):
            for t2 in range(2):
                nc.sync.dma_start(
                    xt_tiles[sb][:, 4 * t2:4 * (t2 + 1), :],
                    xT[512 * t2:512 * (t2 + 1), 512 * sb:512 * (sb + 1)]
                    .rearrange("(i p) e -> p i e", p=128))

        nc.gpsimd.memset(vhat[:, :, :, 0:1], 1.0)
        nc.gpsimd.memset(vhat[:, :, :, 1:DH], 0.0)
        # exp skips the fully-masked prefix of diagonal tiles; the mask
        # multiply zeroes those bytes, which needs them finite (0*NaN=NaN)
        for hp in range(2):
            for h in range(2):
                nc.gpsimd.memset(phat[hp][h][:], 0.0)

        def emit_qk(sb):
            """QK projections for s-block sb (po psum ring)."""
            xt = xt_tiles[sb]
            sl = slice(512 * sb, 512 * (sb + 1))
            for hp in range(2):
                pqk = ps_po.tile([128, 1024], F32, tag="po", name=f"pqk{sb}_{hp}")
                for dt in range(ND):
                    for e in range(2):
                        et = 2 * hp + e
                        nc.tensor.matmul(
                            pqk[:, 512 * e:512 * (e + 1)],
                            wqk_sb[:, dt, 128 * et:128 * (et + 1)],
                            xt[:, dt, :],
                            start=(dt == 0), stop=(dt == ND - 1),
                        )
                    if dt == 3:
                        yield
                nc.vector.tensor_scalar_add(
                    qt_sb[hp][:, sl], pqk[:, 0:512], bqk_sb[:, 2 * hp:2 * hp + 1])
                nc.vector.tensor_scalar_add(
                    kt_sb[hp][:, sl], pqk[:, 512:1024], bqk_sb[:, 2 * hp + 1:2 * hp + 2])
                yield

        def emit_v(sb):
            """V projection for s-block sb (po-ring psum slot, 2 banks)."""
            xt = xt_tiles[sb]
            pv = ps_po.tile([128, 1024], F32, tag="po", name=f"pv{sb}")
            for dt in range(ND):
                for st in range(4):
                    # start only on the first region of each psum bank: start
                    # zeroes the whole 2KB-aligned region, so odd-st halves
                    # accumulate onto the pending-zero left by even st
                    nc.tensor.matmul(
                        pv[:, 256 * st:256 * (st + 1)],
                        xt[:, dt, 128 * st:128 * (st + 1)],
                        wv_sb[:, dt, :],
                        start=(dt == 0 and st % 2 == 0), stop=False,
                    )
                if dt == 3:
                    yield
            for st in range(4):
                # stop only on the second half of each 2KB psum bank: stop
                # closes the whole bank's accumulation group
                nc.tensor.matmul(
                    pv[:, 256 * st:256 * (st + 1)],
                    ones_col[:, 0:128],
                    bv_sb[:],
                    start=False, stop=(st % 2 == 1),
                )
            for st in range(4):
                nc.scalar.copy(
                    vhat[:, :, 4 * sb + st, DH:2 * DH],
                    pv[:, 256 * st:256 * (st + 1)].rearrange("p (h e) -> p h e", h=HC),
                )
            yield

        def emit_attn(hp, qt):
            ph = phat[hp]
            """Scores+exp+PV for head-pair hp, q-block qt; yields after each
            pipeline step so the caller can weave in filler PE work."""
            nkt = 4 * (qt + 1)
            ngrp = nkt // 2
            po = ps_po.tile([128, 2, 512], F32, tag="po", name=f"po{hp}_{qt}")

            def emit_scores(g):
                # 1-bank psc tiles per (h, kt): exp consumes and frees each
                # half as soon as its matmul stops, so the other head-pair's
                # scores never wait on a whole-group drain
                o0 = 2 * g - 4 * qt  # diag-band offset of tile 2g (>=0 in band)
                for h in range(2):
                    sl = slice(64 * h, 64 * (h + 1))
                    psc = [ps_sc.tile([128, 512], F32, tag="sc",
                                      name=f"sc{hp}_{qt}_{g}_{h}_{j}")
                           for j in range(2)]
                    for j in range(2):
                        kt = 2 * g + j
                        nc.tensor.matmul(
                            psc[j][:],
                            kt_sb[hp][sl, 128 * kt:128 * (kt + 1)],
                            qt_sb[hp][sl, 512 * qt:512 * (qt + 1)],
                            start=True, stop=True,
                        )
                    for j in range(2):
                        if o0 < 0:
                            nc.scalar.activation(
                                ph[h][:, 2 * g + j, :], psc[j][:],
                                EXP, scale=0.125,
                            )
                        else:
                            nc.scalar.activation(
                                ph[h][:, 2 * g + j, 128 * (o0 + j):512],
                                psc[j][:, 128 * (o0 + j):512],
                                EXP, scale=0.125,
                            )
                            # per-tile mask so PV of tile 2g needn't wait
                            # for tile 2g+1's exp
                            nc.vector.tensor_mul(
                                ph[h][:, 2 * g + j, :],
                                ph[h][:, 2 * g + j, :],
                                masks_sb[:, o0 + j, :])

            def emit_pv(g, h):
                for j in range(2):
                    kt = 2 * g + j
                    nc.tensor.matmul(
                        po[:, h, :],
                        vhat[:, 2 * hp + h, kt, :],
                        ph[h][:, kt, :],
                        start=(kt == 0), stop=(kt == nkt - 1),
                    )

            # software pipeline: pv lags scores by 2 groups
            for g in range(ngrp):
                emit_scores(g)
                yield "grp"
                if g >= 2:
                    emit_pv(g - 2, 0)
                    emit_pv(g - 2, 1)
                    yield "grp"
            for g in range(max(0, ngrp - 2), ngrp):
                emit_pv(g, 0)
                emit_pv(g, 1)
                yield "tail"

            # normalization, no transposes: 1/den via fast-approx reciprocal
            # (DVE, free-size-bound), po copied to bf16 (ACT) freeing the psum
            # slot early, 1/den broadcast across the 64 feature partitions on
            # the idle GpSimd engine, then one scale+cast mul per head into
            # the proj lhsT layout.
            rec = norm_pool.tile([1, 2, 512], F32, tag="rec")
            pob = norm_pool.tile([DH, 2, 512], BF16, tag="pob")
            recb = norm_pool.tile([64, 2, 512], F32, tag="recb")
            nc.vector.reciprocal_approx_fast(rec[:], po[0:1, :, :])
            nc.scalar.copy(pob[:], po[DH:2 * DH, :, :])
            nc.gpsimd.partition_broadcast(recb[:], rec[:])
            yield "tail"
            sl = slice(512 * qt, 512 * (qt + 1))
            for h in range(2):
                nc.vector.tensor_mul(
                    ot_sb[64 * h:64 * (h + 1), hp, sl], pob[:, h, :], recb[:, h, :])
                yield "norm"

        def emit_proj(qt, tail=False):
            """Projection for q-block qt (after both head-pairs normalized)."""
            for st in range(4):
                s0 = 512 * qt + 128 * st
                py = [ps_sc.tile([128, 512], F32, tag="sc", name=f"py{qt}_{st}_{nt}")
                      for nt in range(2)]
                # ft-outer: consecutive matmuls share the stationary ot tile,
                # skipping a ~100ns weight reload per matmul
                for ft in range(2):
                    for nt in range(2):
                        nc.tensor.matmul(
                            py[nt][:],
                            ot_sb[:, ft, s0:s0 + 128],
                            wp_sb[:, ft, 512 * nt:512 * (nt + 1)],
                            start=(ft == 0), stop=(ft == 1),
                        )
                ysb = ysb_pool.tile([128, 1024], BF16, tag="ysb", name=f"ysb{qt}_{st}")
                eng = nc.scalar.copy if (tail and st % 2 == 1) else nc.vector.tensor_copy
                eng(ysb[:, 0:512], py[0][:])
                yield
                eng(ysb[:, 512:1024], py[1][:])
                nc.sync.dma_start(y[s0:s0 + 128, :], ysb[:])
                yield

        # ---------------- schedule ----------------
        # All QKV upfront (PE-dense stream), then attention q-blocks in
        # DESCENDING order: the long streams (qt=3,2) run first with no
        # filler needed, and the short exp-latency-bound blocks (qt=1,0)
        # at the end are filled by the accumulated projection work.
        for sb in range(NQ):
            for _ in emit_qk(sb):
                pass
            for _ in emit_v(sb):
                pass

        fillerP = []

        def drain(queue, n=-1):
            while queue and n != 0:
                try:
                    next(queue[0])
                    n -= 1
                except StopIteration:
                    queue.pop(0)

        for qt in (3, 2, 1, 0):
            gens = [emit_attn(0, qt), emit_attn(1, qt)]
            ticks = 0
            while gens:
                for g in list(gens):
                    try:
                        next(g)
                    except StopIteration:
                        gens.remove(g)
                        continue
                    ticks += 1
                    # in the last q-block, ration the proj filler so some
                    # remains to cover the final normalization latency
                    if qt > 0 or ticks % 3 == 0:
                        drain(fillerP, 1)
            fillerP.append(emit_proj(qt, tail=(qt == 0)))
        drain(fillerP)

        if dbg is not None:
            dqt, dkt, dvh, dph, dot = dbg
            for hp in range(2):
                nc.sync.dma_start(dqt[hp], qt_sb[hp][:])
                nc.sync.dma_start(dkt[hp], kt_sb[hp][:])
                nc.sync.dma_start(dph[hp], phat[hp][0][:])
            nc.sync.dma_start(dvh[:], vhat[:])
            nc.sync.dma_start(dot[:], ot_sb[:])


_NC = None


def _get_nc():
    global _NC
    if _NC is None:
        _NC = build_nc()
    return _NC


def _make_masks():
    i = np.arange(128)[:, None]
    j = np.arange(512)[None, :]
    m = np.stack([(i + 128 * o <= j) for o in range(4)], axis=1)  # [128, 4, 512]
    return m.astype(ml_dtypes.bfloat16)


def _in_maps(x, W_attn, b_attn, W_proj):
    masks = _make_masks()
    bf = ml_dtypes.bfloat16
    maps = []
    for c in range(8):
        b, g = c // 4, c % 4
        heads = [4 * g + i for i in range(HC)]
        qc = [W_attn[:, 64 * h:64 * (h + 1)] for h in heads]
        kc = [W_attn[:, D + 64 * h:D + 64 * (h + 1)] for h in heads]
        vc = [W_attn[:, 2 * D + 64 * h:2 * D + 64 * (h + 1)] for h in heads]
        bq = [b_attn[64 * h:64 * (h + 1)] for h in heads]
        bk = [b_attn[D + 64 * h:D + 64 * (h + 1)] for h in heads]
        bvs = [b_attn[2 * D + 64 * h:2 * D + 64 * (h + 1)] for h in heads]
        # et order: 0 -> Q hp0, 1 -> K hp0, 2 -> Q hp1, 3 -> K hp1
        wqk_c = np.ascontiguousarray(np.concatenate(
            [qc[0], qc[1], kc[0], kc[1], qc[2], qc[3], kc[2], kc[3]], axis=1)).astype(bf)
        bqk_c = np.concatenate(
            [bq[0], bq[1], bk[0], bk[1], bq[2], bq[3], bk[2], bk[3]])
        bqk_c = np.ascontiguousarray(bqk_c.reshape(4, 128).T).astype(np.float32)
        wv_c = np.ascontiguousarray(np.concatenate(vc, axis=1)).astype(bf)
        bv_c = np.ascontiguousarray(np.concatenate(bvs).reshape(1, EV)).astype(bf)
        wp_c = np.ascontiguousarray(W_proj[EV * g:EV * (g + 1), :]).astype(bf)
        xT_c = np.ascontiguousarray(x[b].T).astype(bf)
        maps.append({
            "xT": xT_c, "wqk": wqk_c, "bqk": bqk_c, "wv": wv_c,
            "bv": bv_c, "wp": wp_c, "masks": masks,
        })
    return maps


def _gather(results, b_proj):
    y = np.empty((B, S, D), np.float32)
    for b in range(B):
        acc = results[4 * b]["y"].astype(np.float32)
        for g in range(1, 4):
            acc = acc + results[4 * b + g]["y"].astype(np.float32)
        y[b] = acc + b_proj[None, :]
    return y


def run(x, W_attn, b_attn, W_proj, b_proj, trace=False):
    x = np.asarray(x, np.float32)
    W_attn = np.asarray(W_attn, np.float32)
    b_attn = np.asarray(b_attn, np.float32)
    W_proj = np.asarray(W_proj, np.float32)
    b_proj = np.asarray(b_proj, np.float32)
    nc = _get_nc()
    res = run_bass_kernel_spmd(nc, _in_maps(x, W_attn, b_attn, W_proj),
                               core_ids=list(range(8)), trace=trace)
    return _gather(res.results, b_proj), res


def kernel(x, W_attn, b_attn, W_proj, b_proj):
    out, _ = run(x, W_attn, b_attn, W_proj, b_proj)
    return out



# revision 29
# speedup vs baseline: 1.0238x; 1.0049x over previous
"""Causal self-attention (B=2, S=2048, D=1024, H=16, Dh=64) on 8 trn2 cores.

Sharding: data-parallel over batch (2 groups of 4 cores) x tensor-parallel over
heads (4 heads/core). Each core computes its heads' attention and a partial
c_proj product; the host sums the 4 bf16 partials per batch (in f32) and adds
b_proj.

All matmuls are bf16 (PSUM accumulates f32; fp8 was tested on HW and is
numerically dead here: e4m3 adds ~2.7% Frobenius noise per quantized tensor,
which does not average down with contraction size). Design notes:
  - Schedule: all QKV upfront (PE-dense stream), then attention q-blocks in
    DESCENDING order with the TWO HEAD-PAIRS' pipelines interleaved tick by
    tick (independent work hides exp latency and halves pipeline
    drain/fill boundaries; phat is per head-pair for this). The projection
    of each q-block is drained one tick at a time as filler into the
    following (shorter, exp-latency-bound) q-blocks.
  - The attention-output normalization has no transposes: PV output
    po = o^T [128, 512] per head (row 0 = softmax denominator via a ones
    column in V-hat -- it must sit at psum partition 0 because
    reciprocal_approx_fast misreads psum inputs at non-zero partition
    offsets; V sits at rows 64:127 since psum reads need 32-aligned
    starts). 1/den via reciprocal_approx_fast (DVE), partition_broadcast
    across the 64 feature rows (GpSimd), one scale+cast mul per head (DVE)
    into the proj lhsT layout.
  - Input DMAs are issued in dt-pair batches from both the sync and gpsimd
    queues (descriptor gen is ~0.65us per dma_start and was gating startup).
  - PSUM is exactly 8 banks: scores 2x[128,1024] (4) + po-ring 2x (4).
    Matmul outputs never cross a 2KB bank; stop=True only on the last
    write of each bank (it closes the whole bank's accumulation group).
  - HW timing is noisy (~+-10%): the chip sits in a per-run DVFS state
    (216ns vs 427ns per 512-row matmul); a PE "warmup" cannot lift it.
"""

import os
import sys

for _p in ("/opt/trn_rl_repo", "/root/.axon_site/_ro/trn_rl_repo"):
    if os.path.isdir(_p) and _p not in sys.path:
        sys.path.insert(0, _p)

import numpy as np
import ml_dtypes

import concourse.bacc as bacc
import concourse.tile as tile
from concourse import mybir
from concourse.bass_utils import run_bass_kernel_spmd

F32 = mybir.dt.float32
BF16 = mybir.dt.bfloat16
EXP = mybir.ActivationFunctionType.Exp

B, S, D, H, DH = 2, 2048, 1024, 16, 64
HC = 4          # heads per core
EQK = 512       # q+k weight cols per core
EV = 256        # v weight cols per core
ND = D // 128   # 8 d-tiles
NS = S // 128   # 16 k-tiles of 128
NQ = S // 512   # 4 s/q-blocks of 512


def build_nc():
    nc = bacc.Bacc("TRN2", target_bir_lowering=False, debug=False)

    xT = nc.dram_tensor("xT", [D, S], BF16, kind="ExternalInput").ap()
    wqk = nc.dram_tensor("wqk", [D, EQK], BF16, kind="ExternalInput").ap()
    bqk = nc.dram_tensor("bqk", [128, 4], F32, kind="ExternalInput").ap()
    wv = nc.dram_tensor("wv", [D, EV], BF16, kind="ExternalInput").ap()
    bv = nc.dram_tensor("bv", [1, EV], BF16, kind="ExternalInput").ap()
    wp = nc.dram_tensor("wp", [EV, D], BF16, kind="ExternalInput").ap()
    masks = nc.dram_tensor("masks", [128, 4, 512], BF16, kind="ExternalInput").ap()
    y = nc.dram_tensor("y", [S, D], BF16, kind="ExternalOutput").ap()

    with tile.TileContext(nc) as tc:
        _emit(nc, tc, xT, wqk, bqk, wv, bv, wp, masks, y)
    nc.compile()
    return nc


def _emit(nc, tc, xT, wqk, bqk, wv, bv, wp, masks, y, dbg=None):
    from contextlib import ExitStack

    with ExitStack() as top:
        consts = top.enter_context(tc.tile_pool(name="consts", bufs=1))
        acts = top.enter_context(tc.tile_pool(name="acts", bufs=1))
        xt_pool = top.enter_context(tc.tile_pool(name="xt", bufs=2))
        ps_sc = top.enter_context(tc.tile_pool(name="ps_sc", bufs=4, space="PSUM"))
        ps_po = top.enter_context(tc.tile_pool(name="ps_po", bufs=2, space="PSUM"))
        norm_pool = top.enter_context(tc.tile_pool(name="norm", bufs=2))
        ysb_pool = top.enter_context(tc.tile_pool(name="ysb", bufs=3))

        ones_col = consts.tile([1, 128], BF16)
        nc.vector.memset(ones_col, 1.0)
        masks_sb = consts.tile([128, 4, 512], BF16)
        bqk_sb = consts.tile([128, 4], F32)
        bv_sb = consts.tile([1, EV], BF16)
        wqk_sb = consts.tile([128, ND, EQK], BF16)
        wv_sb = consts.tile([128, ND, EV], BF16)
        wp_sb = consts.tile([128, 2, D], BF16)

        # persistent activations
        qt_sb = [acts.tile([128, S], BF16, tag=f"qt{hp}", name=f"qt{hp}") for hp in range(2)]
        kt_sb = [acts.tile([128, S], BF16, tag=f"kt{hp}", name=f"kt{hp}") for hp in range(2)]
        # [1 | 0-pad | V]: col 0 = ones (softmax denominator via PV matmul;
        # it must land on psum partition 0 -- reciprocal_approx_fast misreads
        # psum inputs at non-zero partition offsets), V at cols 64:128 so the
        # o rows sit at psum partition 64 (psum reads need 32-aligned starts)
        vhat = acts.tile([128, HC, NS, 128], BF16, tag="vhat")
        phat = [[acts.tile([128, NS, 512], BF16, tag=f"phat{hp}_{h}", name=f"phat{hp}_{h}")
                 for h in range(2)] for hp in range(2)]
        ot_sb = acts.tile([128, 2, S], BF16, tag="ot_sb")  # proj lhsT [feat, s]

        xt_tiles = [xt_pool.tile([128, ND, 512], BF16, tag="xt", name=f"xt{sb}")
                    for sb in range(NQ)]

        # Startup is DMA-descriptor-issue-bound (~0.65us per dma_start on a
        # sequencer). Issue the first QKV block's inputs first, batched in
        # dt-pairs, split across the sync AND gpsimd queues so descriptor
        # generation runs in parallel; everything else in first-use order.
        def pair_dma(eng, dst, src, t2, c0=None, c1=None):
            s = src[256 * t2:256 * (t2 + 1), :] if c0 is None else \
                src[256 * t2:256 * (t2 + 1), c0:c1]
            eng.dma_start(dst[:, 2 * t2:2 * (t2 + 1), :],
                          s.rearrange("(i p) e -> p i e", p=128))

        # first QKV block's inputs: 4 engine queues generate descriptors in
        # parallel so the first dt-pairs all land together
        pair_dma(nc.sync, wqk_sb, wqk, 0)
        pair_dma(nc.gpsimd, xt_tiles[0], xT, 0, 0, 512)
        pair_dma(nc.scalar, wqk_sb, wqk, 1)
        pair_dma(nc.scalar, xt_tiles[0], xT, 1, 0, 512)
        for t2 in range(2, 4):
            pair_dma(nc.sync, wqk_sb, wqk, t2)
            pair_dma(nc.gpsimd, xt_tiles[0], xT, t2, 0, 512)
        for t2 in range(2):
            nc.sync.dma_start(
                wv_sb[:, 4 * t2:4 * (t2 + 1), :],
                wv[512 * t2:512 * (t2 + 1), :].rearrange("(i p) e -> p i e", p=128))
        nc.gpsimd.dma_start(bqk_sb[:], bqk[:])
        nc.gpsimd.dma_start(bv_sb[:], bv[:])
        for t2 in range(4):
            pair_dma(nc.sync, xt_tiles[1], xT, t2, 512, 1024)
        nc.gpsimd.dma_start(
            wp_sb[:], wp[:].rearrange("(i p) e -> p i e", p=128))
        nc.sync.dma_start(masks_sb[:], masks[:])
        for sb in (2, 3):
            for t2 in range(2):
                nc.sync.dma_start(
                    xt_tiles[sb][:, 4 * t2:4 * (t2 + 1), :],
                    xT[512 * t2:512 * (t2 + 1), 512 * sb:512 * (sb + 1)]
                    .rearrange("(i p) e -> p i e", p=128))

        nc.gpsimd.memset(vhat[:, :, :, 0:1], 1.0)
        nc.gpsimd.memset(vhat[:, :, :, 1:DH], 0.0)
        # exp skips the fully-masked prefix of diagonal tiles; the mask
        # multiply zeroes those bytes, which needs them finite (0*NaN=NaN)
        for hp in range(2):
            for h in range(2):
                nc.gpsimd.memset(phat[hp][h][:], 0.0)

        def emit_qk(sb):
            """QK projections for s-block sb (po psum ring)."""
            xt = xt_tiles[sb]
            sl = slice(512 * sb, 512 * (sb + 1))
            for hp in range(2):
                pqk = ps_po.tile([128, 1024], F32, tag="po", name=f"pqk{sb}_{hp}")
                for dt in range(ND):
                    for e in range(2):
                        et = 2 * hp + e
                        nc.tensor.matmul(
                            pqk[:, 512 * e:512 * (e + 1)],
                            wqk_sb[:, dt, 128 * et:128 * (et + 1)],
                            xt[:, dt, :],
                            start=(dt == 0), stop=(dt == ND - 1),
                        )
                    if dt == 3:
                        yield
                nc.vector.tensor_scalar_add(
                    qt_sb[hp][:, sl], pqk[:, 0:512], bqk_sb[:, 2 * hp:2 * hp + 1])
                nc.vector.tensor_scalar_add(
                    kt_sb[hp][:, sl], pqk[:, 512:1024], bqk_sb[:, 2 * hp + 1:2 * hp + 2])
                yield

        def emit_v(sb):
            """V projection for s-block sb (po-ring psum slot, 2 banks)."""
            xt = xt_tiles[sb]
            pv = ps_po.tile([128, 1024], F32, tag="po", name=f"pv{sb}")
            for dt in range(ND):
                for st in range(4):
                    # start only on the first region of each psum bank: start
                    # zeroes the whole 2KB-aligned region, so odd-st halves
                    # accumulate onto the pending-zero left by even st
                    nc.tensor.matmul(
                        pv[:, 256 * st:256 * (st + 1)],
                        xt[:, dt, 128 * st:128 * (st + 1)],
                        wv_sb[:, dt, :],
                        start=(dt == 0 and st % 2 == 0), stop=False,
                    )
                if dt == 3:
                    yield
            for st in range(4):
                # stop only on the second half of each 2KB psum bank: stop
                # closes the whole bank's accumulation group
                nc.tensor.matmul(
                    pv[:, 256 * st:256 * (st + 1)],
                    ones_col[:, 0:128],
                    bv_sb[:],
                    start=False, stop=(st % 2 == 1),
                )
            for st in range(4):
                nc.scalar.copy(
                    vhat[:, :, 4 * sb + st, DH:2 * DH],
                    pv[:, 256 * st:256 * (st + 1)].rearrange("p (h e) -> p h e", h=HC),
                )
            yield

        def emit_attn(hp, qt):
            ph = phat[hp]
            """Scores+exp+PV for head-pair hp, q-block qt; yields after each
            pipeline step so the caller can weave in filler PE work."""
            nkt = 4 * (qt + 1)
            ngrp = nkt // 2
            po = ps_po.tile([128, 2, 512], F32, tag="po", name=f"po{hp}_{qt}")

            def emit_scores(g):
                # 1-bank psc tiles per (h, kt): exp consumes and frees each
                # half as soon as its matmul stops, so the other head-pair's
                # scores never wait on a whole-group drain
                o0 = 2 * g - 4 * qt  # diag-band offset of tile 2g (>=0 in band)
                for h in range(2):
                    sl = slice(64 * h, 64 * (h + 1))
                    psc = [ps_sc.tile([128, 512], F32, tag="sc",
                                      name=f"sc{hp}_{qt}_{g}_{h}_{j}")
                           for j in range(2)]
                    for j in range(2):
                        kt = 2 * g + j
                        nc.tensor.matmul(
                            psc[j][:],
                            kt_sb[hp][sl, 128 * kt:128 * (kt + 1)],
                            qt_sb[hp][sl, 512 * qt:512 * (qt + 1)],
                            start=True, stop=True,
                        )
                    for j in range(2):
                        if o0 < 0:
                            nc.scalar.activation(
                                ph[h][:, 2 * g + j, :], psc[j][:],
                                EXP, scale=0.125,
                            )
                        else:
                            nc.scalar.activation(
                                ph[h][:, 2 * g + j, 128 * (o0 + j):512],
                                psc[j][:, 128 * (o0 + j):512],
                                EXP, scale=0.125,
                            )
                            # per-tile mask so PV of tile 2g needn't wait
                            # for tile 2g+1's exp
                            nc.vector.tensor_mul(
                                ph[h][:, 2 * g + j, :],
                                ph[h][:, 2 * g + j, :],
                                masks_sb[:, o0 + j, :])

            def emit_pv(g, h):
                for j in range(2):
                    kt = 2 * g + j
                    nc.tensor.matmul(
                        po[:, h, :],
                        vhat[:, 2 * hp + h, kt, :],
                        ph[h][:, kt, :],
                        start=(kt == 0), stop=(kt == nkt - 1),
                    )

            # software pipeline: pv lags scores by 2 groups
            for g in range(ngrp):
                emit_scores(g)
                yield "grp"
                if g >= 2:
                    emit_pv(g - 2, 0)
                    emit_pv(g - 2, 1)
                    yield "grp"
            for g in range(max(0, ngrp - 2), ngrp):
                emit_pv(g, 0)
                emit_pv(g, 1)
                yield "tail"

            # normalization, no transposes: 1/den via fast-approx reciprocal
            # (DVE, free-size-bound), po copied to bf16 (ACT) freeing the psum
            # slot early, 1/den broadcast across the 64 feature partitions on
            # the idle GpSimd engine, then one scale+cast mul per head into
            # the proj lhsT layout.
            rec = norm_pool.tile([1, 2, 512], F32, tag="rec")
            pob = norm_pool.tile([DH, 2, 512], BF16, tag="pob")
            recb = norm_pool.tile([64, 2, 512], F32, tag="recb")
            nc.vector.reciprocal_approx_fast(rec[:], po[0:1, :, :])
            nc.scalar.copy(pob[:], po[DH:2 * DH, :, :])
            nc.gpsimd.partition_broadcast(recb[:], rec[:])
            yield "tail"
            sl = slice(512 * qt, 512 * (qt + 1))
            for h in range(2):
                nc.vector.tensor_mul(
                    ot_sb[64 * h:64 * (h + 1), hp, sl], pob[:, h, :], recb[:, h, :])
                yield "norm"

        def emit_proj(qt, tail=False):
            """Projection for q-block qt (after both head-pairs normalized)."""
            for st in range(4):
                s0 = 512 * qt + 128 * st
                py = [ps_sc.tile([128, 512], F32, tag="sc", name=f"py{qt}_{st}_{nt}")
                      for nt in range(2)]
                # ft-outer: consecutive matmuls share the stationary ot tile,
                # skipping a ~100ns weight reload per matmul
                for ft in range(2):
                    for nt in range(2):
                        nc.tensor.matmul(
                            py[nt][:],
                            ot_sb[:, ft, s0:s0 + 128],
                            wp_sb[:, ft, 512 * nt:512 * (nt + 1)],
                            start=(ft == 0), stop=(ft == 1),
                        )
                ysb = ysb_pool.tile([128, 1024], BF16, tag="ysb", name=f"ysb{qt}_{st}")
                eng = nc.scalar.copy if (tail and st % 2 == 1) else nc.vector.tensor_copy
                eng(ysb[:, 0:512], py[0][:])
                yield
                eng(ysb[:, 512:1024], py[1][:])
                nc.sync.dma_start(y[s0:s0 + 128, :], ysb[:])
                yield

        # ---------------- schedule ----------------
        # All QKV upfront (PE-dense stream), then attention q-blocks in
        # DESCENDING order: the long streams (qt=3,2) run first with no
        # filler needed, and the short exp-latency-bound blocks (qt=1,0)
        # at the end are filled by the accumulated projection work.
        for sb in range(NQ):
            for _ in emit_qk(sb):
                pass
            for _ in emit_v(sb):
                pass

        fillerP = []

        def drain(queue, n=-1):
            while queue and n != 0:
                try:
                    next(queue[0])
                    n -= 1
                except StopIteration:
                    queue.pop(0)

        for qt in (3, 2, 1, 0):
            gens = [emit_attn(0, qt), emit_attn(1, qt)]
            ticks = 0
            while gens:
                for g in list(gens):
                    try:
                        next(g)
                    except StopIteration:
                        gens.remove(g)
                        continue
                    ticks += 1
                    # in the last q-block, ration the proj filler so some
                    # remains to cover the final normalization latency
                    if qt > 0 or ticks % 3 == 0:
                        drain(fillerP, 1)
            fillerP.append(emit_proj(qt, tail=(qt == 0)))
        drain(fillerP)

        if dbg is not None:
            dqt, dkt, dvh, dph, dot = dbg
            for hp in range(2):
                nc.sync.dma_start(dqt[hp], qt_sb[hp][:])
                nc.sync.dma_start(dkt[hp], kt_sb[hp][:])
                nc.sync.dma_start(dph[hp], phat[hp][0][:])
            nc.sync.dma_start(dvh[:], vhat[:])
            nc.sync.dma_start(dot[:], ot_sb[:])


_NC = None


def _get_nc():
    global _NC
    if _NC is None:
        _NC = build_nc()
    return _NC


def _make_masks():
    i = np.arange(128)[:, None]
    j = np.arange(512)[None, :]
    m = np.stack([(i + 128 * o <= j) for o in range(4)], axis=1)  # [128, 4, 512]
    return m.astype(ml_dtypes.bfloat16)


def _in_maps(x, W_attn, b_attn, W_proj):
    masks = _make_masks()
    bf = ml_dtypes.bfloat16
    maps = []
    for c in range(8):
        b, g = c // 4, c % 4
        heads = [4 * g + i for i in range(HC)]
        qc = [W_attn[:, 64 * h:64 * (h + 1)] for h in heads]
        kc = [W_attn[:, D + 64 * h:D + 64 * (h + 1)] for h in heads]
        vc = [W_attn[:, 2 * D + 64 * h:2 * D + 64 * (h + 1)] for h in heads]
        bq = [b_attn[64 * h:64 * (h + 1)] for h in heads]
        bk = [b_attn[D + 64 * h:D + 64 * (h + 1)] for h in heads]
        bvs = [b_attn[2 * D + 64 * h:2 * D + 64 * (h + 1)] for h in heads]
        # et order: 0 -> Q hp0, 1 -> K hp0, 2 -> Q hp1, 3 -> K hp1
        wqk_c = np.ascontiguousarray(np.concatenate(
            [qc[0], qc[1], kc[0], kc[1], qc[2], qc[3], kc[2], kc[3]], axis=1)).astype(bf)
        bqk_c = np.concatenate(
            [bq[0], bq[1], bk[0], bk[1], bq[2], bq[3], bk[2], bk[3]])
        bqk_c = np.ascontiguousarray(bqk_c.reshape(4, 128).T).astype(np.float32)
        wv_c = np.ascontiguousarray(np.concatenate(vc, axis=1)).astype(bf)
        bv_c = np.ascontiguousarray(np.concatenate(bvs).reshape(1, EV)).astype(bf)
        wp_c = np.ascontiguousarray(W_proj[EV * g:EV * (g + 1), :]).astype(bf)
        xT_c = np.ascontiguousarray(x[b].T).astype(bf)
        maps.append({
            "xT": xT_c, "wqk": wqk_c, "bqk": bqk_c, "wv": wv_c,
            "bv": bv_c, "wp": wp_c, "masks": masks,
        })
    return maps


def _gather(results, b_proj):
    y = np.empty((B, S, D), np.float32)
    for b in range(B):
        acc = results[4 * b]["y"].astype(np.float32)
        for g in range(1, 4):
            acc = acc + results[4 * b + g]["y"].astype(np.float32)
        y[b] = acc + b_proj[None, :]
    return y


def run(x, W_attn, b_attn, W_proj, b_proj, trace=False):
    x = np.asarray(x, np.float32)
    W_attn = np.asarray(W_attn, np.float32)
    b_attn = np.asarray(b_attn, np.float32)
    W_proj = np.asarray(W_proj, np.float32)
    b_proj = np.asarray(b_proj, np.float32)
    nc = _get_nc()
    res = run_bass_kernel_spmd(nc, _in_maps(x, W_attn, b_attn, W_proj),
                               core_ids=list(range(8)), trace=trace)
    return _gather(res.results, b_proj), res


def kernel(x, W_attn, b_attn, W_proj, b_proj):
    out, _ = run(x, W_attn, b_attn, W_proj, b_proj)
    return out



# revision 30
# speedup vs baseline: 1.0293x; 1.0053x over previous
"""Causal self-attention (B=2, S=2048, D=1024, H=16, Dh=64) on 8 trn2 cores.

Sharding: data-parallel over batch (2 groups of 4 cores) x tensor-parallel over
heads (4 heads/core). Each core computes its heads' attention and a partial
c_proj product; the host sums the 4 bf16 partials per batch (in f32) and adds
b_proj.

All matmuls are bf16 (PSUM accumulates f32; fp8 was tested on HW and is
numerically dead here: e4m3 adds ~2.7% Frobenius noise per quantized tensor,
which does not average down with contraction size). Design notes:
  - Schedule: all QKV upfront (PE-dense stream), then attention q-blocks in
    DESCENDING order with the TWO HEAD-PAIRS' pipelines interleaved tick by
    tick (independent work hides exp latency and halves pipeline
    drain/fill boundaries; phat is per head-pair for this). The projection
    of each q-block is drained one tick at a time as filler into the
    following (shorter, exp-latency-bound) q-blocks.
  - The attention-output normalization has no transposes: PV output
    po = o^T [128, 512] per head (row 0 = softmax denominator via a ones
    column in V-hat -- it must sit at psum partition 0 because
    reciprocal_approx_fast misreads psum inputs at non-zero partition
    offsets; V sits at rows 64:127 since psum reads need 32-aligned
    starts). 1/den via reciprocal_approx_fast (DVE), partition_broadcast
    across the 64 feature rows (GpSimd), one scale+cast mul per head (DVE)
    into the proj lhsT layout.
  - Input DMAs are issued in dt-pair batches from both the sync and gpsimd
    queues (descriptor gen is ~0.65us per dma_start and was gating startup).
  - PSUM is exactly 8 banks: scores 2x[128,1024] (4) + po-ring 2x (4).
    Matmul outputs never cross a 2KB bank; stop=True only on the last
    write of each bank (it closes the whole bank's accumulation group).
  - HW timing is noisy (~+-10%): the chip sits in a per-run DVFS state
    (216ns vs 427ns per 512-row matmul); a PE "warmup" cannot lift it.
"""

import os
import sys

for _p in ("/opt/trn_rl_repo", "/root/.axon_site/_ro/trn_rl_repo"):
    if os.path.isdir(_p) and _p not in sys.path:
        sys.path.insert(0, _p)

import numpy as np
import ml_dtypes

import concourse.bacc as bacc
import concourse.tile as tile
from concourse import mybir
from concourse.bass_utils import run_bass_kernel_spmd

F32 = mybir.dt.float32
BF16 = mybir.dt.bfloat16
EXP = mybir.ActivationFunctionType.Exp

B, S, D, H, DH = 2, 2048, 1024, 16, 64
HC = 4          # heads per core
EQK = 512       # q+k weight cols per core
EV = 256        # v weight cols per core
ND = D // 128   # 8 d-tiles
NS = S // 128   # 16 k-tiles of 128
NQ = S // 512   # 4 s/q-blocks of 512


def build_nc():
    nc = bacc.Bacc("TRN2", target_bir_lowering=False, debug=False)

    xT = nc.dram_tensor("xT", [D, S], BF16, kind="ExternalInput").ap()
    wqk = nc.dram_tensor("wqk", [D, EQK], BF16, kind="ExternalInput").ap()
    bqk = nc.dram_tensor("bqk", [128, 4], F32, kind="ExternalInput").ap()
    wv = nc.dram_tensor("wv", [D, EV], BF16, kind="ExternalInput").ap()
    bv = nc.dram_tensor("bv", [1, EV], BF16, kind="ExternalInput").ap()
    wp = nc.dram_tensor("wp", [EV, D], BF16, kind="ExternalInput").ap()
    masks = nc.dram_tensor("masks", [128, 4, 512], BF16, kind="ExternalInput").ap()
    y = nc.dram_tensor("y", [S, D], BF16, kind="ExternalOutput").ap()

    with tile.TileContext(nc) as tc:
        _emit(nc, tc, xT, wqk, bqk, wv, bv, wp, masks, y)
    nc.compile()
    return nc


def _emit(nc, tc, xT, wqk, bqk, wv, bv, wp, masks, y, dbg=None):
    from contextlib import ExitStack

    with ExitStack() as top:
        consts = top.enter_context(tc.tile_pool(name="consts", bufs=1))
        acts = top.enter_context(tc.tile_pool(name="acts", bufs=1))
        xt_pool = top.enter_context(tc.tile_pool(name="xt", bufs=2))
        ps_sc = top.enter_context(tc.tile_pool(name="ps_sc", bufs=4, space="PSUM"))
        ps_po = top.enter_context(tc.tile_pool(name="ps_po", bufs=2, space="PSUM"))
        norm_pool = top.enter_context(tc.tile_pool(name="norm", bufs=2))
        ysb_pool = top.enter_context(tc.tile_pool(name="ysb", bufs=3))

        ones_col = consts.tile([1, 128], BF16)
        nc.vector.memset(ones_col, 1.0)
        masks_sb = consts.tile([128, 4, 512], BF16)
        bqk_sb = consts.tile([128, 4], F32)
        bv_sb = consts.tile([1, EV], BF16)
        wqk_sb = consts.tile([128, ND, EQK], BF16)
        wv_sb = consts.tile([128, ND, EV], BF16)
        wp_sb = consts.tile([128, 2, D], BF16)

        # persistent activations
        qt_sb = [acts.tile([128, S], BF16, tag=f"qt{hp}", name=f"qt{hp}") for hp in range(2)]
        kt_sb = [acts.tile([128, S], BF16, tag=f"kt{hp}", name=f"kt{hp}") for hp in range(2)]
        # [1 | 0-pad | V]: col 0 = ones (softmax denominator via PV matmul;
        # it must land on psum partition 0 -- reciprocal_approx_fast misreads
        # psum inputs at non-zero partition offsets), V at cols 64:128 so the
        # o rows sit at psum partition 64 (psum reads need 32-aligned starts)
        vhat = acts.tile([128, HC, NS, 128], BF16, tag="vhat")
        phat = [[acts.tile([128, NS, 512], BF16, tag=f"phat{hp}_{h}", name=f"phat{hp}_{h}")
                 for h in range(2)] for hp in range(2)]
        ot_sb = acts.tile([128, 2, S], BF16, tag="ot_sb")  # proj lhsT [feat, s]

        xt_tiles = [xt_pool.tile([128, ND, 512], BF16, tag="xt", name=f"xt{sb}")
                    for sb in range(NQ)]

        # Startup is DMA-descriptor-issue-bound (~0.65us per dma_start on a
        # sequencer). Issue the first QKV block's inputs first, batched in
        # dt-pairs, split across the sync AND gpsimd queues so descriptor
        # generation runs in parallel; everything else in first-use order.
        def pair_dma(eng, dst, src, t2, c0=None, c1=None):
            s = src[256 * t2:256 * (t2 + 1), :] if c0 is None else \
                src[256 * t2:256 * (t2 + 1), c0:c1]
            eng.dma_start(dst[:, 2 * t2:2 * (t2 + 1), :],
                          s.rearrange("(i p) e -> p i e", p=128))

        # first QKV block's inputs: 4 engine queues generate descriptors in
        # parallel so the first dt-pairs all land together
        pair_dma(nc.sync, wqk_sb, wqk, 0)
        pair_dma(nc.gpsimd, xt_tiles[0], xT, 0, 0, 512)
        pair_dma(nc.scalar, wqk_sb, wqk, 1)
        pair_dma(nc.scalar, xt_tiles[0], xT, 1, 0, 512)
        for t2 in range(2, 4):
            pair_dma(nc.sync, wqk_sb, wqk, t2)
            pair_dma(nc.gpsimd, xt_tiles[0], xT, t2, 0, 512)
        for t2 in range(2):
            nc.sync.dma_start(
                wv_sb[:, 4 * t2:4 * (t2 + 1), :],
                wv[512 * t2:512 * (t2 + 1), :].rearrange("(i p) e -> p i e", p=128))
        nc.gpsimd.dma_start(bqk_sb[:], bqk[:])
        nc.gpsimd.dma_start(bv_sb[:], bv[:])
        for t2 in range(4):
            pair_dma(nc.sync, xt_tiles[1], xT, t2, 512, 1024)
        nc.gpsimd.dma_start(
            wp_sb[:], wp[:].rearrange("(i p) e -> p i e", p=128))
        nc.sync.dma_start(masks_sb[:], masks[:])
        for sb in (2, 3):
            for t2 in range(2):
                nc.sync.dma_start(
                    xt_tiles[sb][:, 4 * t2:4 * (t2 + 1), :],
                    xT[512 * t2:512 * (t2 + 1), 512 * sb:512 * (sb + 1)]
                    .rearrange("(i p) e -> p i e", p=128))

        nc.gpsimd.memset(vhat[:, :, :, 0:1], 1.0)
        nc.gpsimd.memset(vhat[:, :, :, 1:DH], 0.0)
        # exp skips the fully-masked prefix of diagonal tiles; the mask
        # multiply zeroes those bytes, which needs them finite (0*NaN=NaN)
        for hp in range(2):
            for h in range(2):
                nc.gpsimd.memset(phat[hp][h][:], 0.0)

        def emit_qk(sb):
            """QK projections for s-block sb (po psum ring)."""
            xt = xt_tiles[sb]
            sl = slice(512 * sb, 512 * (sb + 1))
            for hp in range(2):
                pqk = ps_po.tile([128, 1024], F32, tag="po", name=f"pqk{sb}_{hp}")
                for dt in range(ND):
                    for e in range(2):
                        et = 2 * hp + e
                        nc.tensor.matmul(
                            pqk[:, 512 * e:512 * (e + 1)],
                            wqk_sb[:, dt, 128 * et:128 * (et + 1)],
                            xt[:, dt, :],
                            start=(dt == 0), stop=(dt == ND - 1),
                        )
                    if dt == 3:
                        yield
                nc.vector.tensor_scalar_add(
                    qt_sb[hp][:, sl], pqk[:, 0:512], bqk_sb[:, 2 * hp:2 * hp + 1])
                nc.vector.tensor_scalar_add(
                    kt_sb[hp][:, sl], pqk[:, 512:1024], bqk_sb[:, 2 * hp + 1:2 * hp + 2])
                yield

        def emit_v(sb):
            """V projection for s-block sb (po-ring psum slot, 2 banks)."""
            xt = xt_tiles[sb]
            pv = ps_po.tile([128, 1024], F32, tag="po", name=f"pv{sb}")
            for dt in range(ND):
                for st in range(4):
                    # start only on the first region of each psum bank: start
                    # zeroes the whole 2KB-aligned region, so odd-st halves
                    # accumulate onto the pending-zero left by even st
                    nc.tensor.matmul(
                        pv[:, 256 * st:256 * (st + 1)],
                        xt[:, dt, 128 * st:128 * (st + 1)],
                        wv_sb[:, dt, :],
                        start=(dt == 0 and st % 2 == 0), stop=False,
                    )
                if dt == 3:
                    yield
            for st in range(4):
                # stop only on the second half of each 2KB psum bank: stop
                # closes the whole bank's accumulation group
                nc.tensor.matmul(
                    pv[:, 256 * st:256 * (st + 1)],
                    ones_col[:, 0:128],
                    bv_sb[:],
                    start=False, stop=(st % 2 == 1),
                )
            for st in range(4):
                nc.scalar.copy(
                    vhat[:, :, 4 * sb + st, DH:2 * DH],
                    pv[:, 256 * st:256 * (st + 1)].rearrange("p (h e) -> p h e", h=HC),
                )
            yield

        def emit_attn(hp, qt):
            ph = phat[hp]
            """Scores+exp+PV for head-pair hp, q-block qt; yields after each
            pipeline step so the caller can weave in filler PE work."""
            nkt = 4 * (qt + 1)
            ngrp = nkt // 2
            po = ps_po.tile([128, 2, 512], F32, tag="po", name=f"po{hp}_{qt}")

            def emit_scores(g):
                # 1-bank psc tiles per (h, kt): exp consumes and frees each
                # half as soon as its matmul stops, so the other head-pair's
                # scores never wait on a whole-group drain
                o0 = 2 * g - 4 * qt  # diag-band offset of tile 2g (>=0 in band)
                for h in range(2):
                    sl = slice(64 * h, 64 * (h + 1))
                    psc = [ps_sc.tile([128, 512], F32, tag="sc",
                                      name=f"sc{hp}_{qt}_{g}_{h}_{j}")
                           for j in range(2)]
                    for j in range(2):
                        kt = 2 * g + j
                        nc.tensor.matmul(
                            psc[j][:],
                            kt_sb[hp][sl, 128 * kt:128 * (kt + 1)],
                            qt_sb[hp][sl, 512 * qt:512 * (qt + 1)],
                            start=True, stop=True,
                        )
                    for j in range(2):
                        if o0 < 0:
                            nc.scalar.activation(
                                ph[h][:, 2 * g + j, :], psc[j][:],
                                EXP, scale=0.125,
                            )
                        else:
                            nc.scalar.activation(
                                ph[h][:, 2 * g + j, 128 * (o0 + j):512],
                                psc[j][:, 128 * (o0 + j):512],
                                EXP, scale=0.125,
                            )
                            # per-tile mask so PV of tile 2g needn't wait
                            # for tile 2g+1's exp
                            nc.vector.tensor_mul(
                                ph[h][:, 2 * g + j, :],
                                ph[h][:, 2 * g + j, :],
                                masks_sb[:, o0 + j, :])

            def emit_pv(g, h):
                for j in range(2):
                    kt = 2 * g + j
                    nc.tensor.matmul(
                        po[:, h, :],
                        vhat[:, 2 * hp + h, kt, :],
                        ph[h][:, kt, :],
                        start=(kt == 0), stop=(kt == nkt - 1),
                    )

            # software pipeline: pv lags scores by 2 groups
            for g in range(ngrp):
                emit_scores(g)
                yield "grp"
                if g >= 2:
                    emit_pv(g - 2, 0)
                    emit_pv(g - 2, 1)
                    yield "grp"
            for g in range(max(0, ngrp - 2), ngrp):
                emit_pv(g, 0)
                emit_pv(g, 1)
                yield "tail"

            # normalization, no transposes: 1/den via fast-approx reciprocal
            # (DVE, free-size-bound), po copied to bf16 (ACT) freeing the psum
            # slot early, 1/den broadcast across the 64 feature partitions on
            # the idle GpSimd engine, then one scale+cast mul per head into
            # the proj lhsT layout.
            rec = norm_pool.tile([1, 2, 512], F32, tag="rec")
            pob = norm_pool.tile([DH, 2, 512], BF16, tag="pob")
            recb = norm_pool.tile([64, 2, 512], F32, tag="recb")
            nc.vector.reciprocal_approx_fast(rec[:], po[0:1, :, :])
            nc.scalar.copy(pob[:], po[DH:2 * DH, :, :])
            nc.gpsimd.partition_broadcast(recb[:], rec[:])
            yield "tail"
            sl = slice(512 * qt, 512 * (qt + 1))
            for h in range(2):
                nc.vector.tensor_mul(
                    ot_sb[64 * h:64 * (h + 1), hp, sl], pob[:, h, :], recb[:, h, :])
                yield "norm"

        def emit_proj(qt, tail=False):
            """Projection for q-block qt (after both head-pairs normalized)."""
            for st in range(4):
                s0 = 512 * qt + 128 * st
                py = [ps_sc.tile([128, 512], F32, tag="sc", name=f"py{qt}_{st}_{nt}")
                      for nt in range(2)]
                # ft-outer: consecutive matmuls share the stationary ot tile,
                # skipping a ~100ns weight reload per matmul
                for ft in range(2):
                    for nt in range(2):
                        nc.tensor.matmul(
                            py[nt][:],
                            ot_sb[:, ft, s0:s0 + 128],
                            wp_sb[:, ft, 512 * nt:512 * (nt + 1)],
                            start=(ft == 0), stop=(ft == 1),
                        )
                ysb = ysb_pool.tile([128, 1024], BF16, tag="ysb", name=f"ysb{qt}_{st}")
                eng = nc.scalar.copy if (tail and st % 2 == 1) else nc.vector.tensor_copy
                eng(ysb[:, 0:512], py[0][:])
                yield
                eng(ysb[:, 512:1024], py[1][:])
                nc.sync.dma_start(y[s0:s0 + 128, :], ysb[:])
                yield

        # ---------------- schedule ----------------
        # All QKV upfront (PE-dense stream), then attention q-blocks in
        # DESCENDING order: the long streams (qt=3,2) run first with no
        # filler needed, and the short exp-latency-bound blocks (qt=1,0)
        # at the end are filled by the accumulated projection work.
        for sb in range(NQ):
            for _ in emit_qk(sb):
                pass
            for _ in emit_v(sb):
                pass

        fillerP = []

        def drain(queue, n=-1):
            while queue and n != 0:
                try:
                    next(queue[0])
                    n -= 1
                except StopIteration:
                    queue.pop(0)

        for qt in (3, 2, 1, 0):
            gens = [emit_attn(0, qt), emit_attn(1, qt)]
            ticks = 0
            while gens:
                for g in list(gens):
                    try:
                        next(g)
                    except StopIteration:
                        gens.remove(g)
                        continue
                    ticks += 1
                    # in the last q-block, ration the proj filler so some
                    # remains to cover the final normalization latency
                    if qt > 0 or ticks % 4 == 0:
                        drain(fillerP, 1)
            fillerP.append(emit_proj(qt, tail=(qt == 0)))
        drain(fillerP)

        if dbg is not None:
            dqt, dkt, dvh, dph, dot = dbg
            for hp in range(2):
                nc.sync.dma_start(dqt[hp], qt_sb[hp][:])
                nc.sync.dma_start(dkt[hp], kt_sb[hp][:])
                nc.sync.dma_start(dph[hp], phat[hp][0][:])
            nc.sync.dma_start(dvh[:], vhat[:])
            nc.sync.dma_start(dot[:], ot_sb[:])


_NC = None


def _get_nc():
    global _NC
    if _NC is None:
        _NC = build_nc()
    return _NC


def _make_masks():
    i = np.arange(128)[:, None]
    j = np.arange(512)[None, :]
    m = np.stack([(i + 128 * o <= j) for o in range(4)], axis=1)  # [128, 4, 512]
    return m.astype(ml_dtypes.bfloat16)


def _in_maps(x, W_attn, b_attn, W_proj):
    masks = _make_masks()
    bf = ml_dtypes.bfloat16
    maps = []
    for c in range(8):
        b, g = c // 4, c % 4
        heads = [4 * g + i for i in range(HC)]
        qc = [W_attn[:, 64 * h:64 * (h + 1)] for h in heads]
        kc = [W_attn[:, D + 64 * h:D + 64 * (h + 1)] for h in heads]
        vc = [W_attn[:, 2 * D + 64 * h:2 * D + 64 * (h + 1)] for h in heads]
        bq = [b_attn[64 * h:64 * (h + 1)] for h in heads]
        bk = [b_attn[D + 64 * h:D + 64 * (h + 1)] for h in heads]
        bvs = [b_attn[2 * D + 64 * h:2 * D + 64 * (h + 1)] for h in heads]
        # et order: 0 -> Q hp0, 1 -> K hp0, 2 -> Q hp1, 3 -> K hp1
        wqk_c = np.ascontiguousarray(np.concatenate(
            [qc[0], qc[1], kc[0], kc[1], qc[2], qc[3], kc[2], kc[3]], axis=1)).astype(bf)
        bqk_c = np.concatenate(
            [bq[0], bq[1], bk[0], bk[1], bq[2], bq[3], bk[2], bk[3]])
        bqk_c = np.ascontiguousarray(bqk_c.reshape(4, 128).T).astype(np.float32)
        wv_c = np.ascontiguousarray(np.concatenate(vc, axis=1)).astype(bf)
        bv_c = np.ascontiguousarray(np.concatenate(bvs).reshape(1, EV)).astype(bf)
        wp_c = np.ascontiguousarray(W_proj[EV * g:EV * (g + 1), :]).astype(bf)
        xT_c = np.ascontiguousarray(x[b].T).astype(bf)
        maps.append({
            "xT": xT_c, "wqk": wqk_c, "bqk": bqk_c, "wv": wv_c,
            "bv": bv_c, "wp": wp_c, "masks": masks,
        })
    return maps


def _gather(results, b_proj):
    y = np.empty((B, S, D), np.float32)
    for b in range(B):
        acc = results[4 * b]["y"].astype(np.float32)
        for g in range(1, 4):
            acc = acc + results[4 * b + g]["y"].astype(np.float32)
        y[b] = acc + b_proj[None, :]
    return y


def run(x, W_attn, b_attn, W_proj, b_proj, trace=False):
    x = np.asarray(x, np.float32)
    W_attn = np.asarray(W_attn, np.float32)
    b_attn = np.asarray(b_attn, np.float32)
    W_proj = np.asarray(W_proj, np.float32)
    b_proj = np.asarray(b_proj, np.float32)
    nc = _get_nc()
    res = run_bass_kernel_spmd(nc, _in_maps(x, W_attn, b_attn, W_proj),
                               core_ids=list(range(8)), trace=trace)
    return _gather(res.results, b_proj), res


def kernel(x, W_attn, b_attn, W_proj, b_proj):
    out, _ = run(x, W_attn, b_attn, W_proj, b_proj)
    return out

